# revision 1
# baseline (speedup 1.0000x reference)
"""CenterLoss kernel for Trainium2 (raw Bass/Bacc, no Tile), 8-core
data-parallel.

Key algebraic insight: the reference builds the full [B, C] squared-
distance matrix and masks it with one-hot(labels), so only
distmat[i, labels[i]] survives.  The loss is therefore

    loss = (1/B) * sum_i || x_i - centers[labels[i]] ||^2
         = (1/B) * [ sum x^2  - 2 sum_i x_i . c_{l_i}  + sum_i ||c_{l_i}||^2 ]

which needs only a gather of each sample's center row (indirect DMA),
not the [4096, 10000] matmul.

v3 design (vs v1's sub/square/PE chain):
  * Inputs staged in bf16 on host (x, centers): halves HBM/DMA traffic
    to ~1 MB per core.  Loss tolerance is 2e-2; measured bf16 error here
    is ~1e-5 (the three partial sums each average out quantization noise).
  * Expansion form, fused one-pass ops with free-dim accumulation:
      Vector : scalar_tensor_tensor (x*-2)*c, accum = -2 sum x.c
      Scalar : ACT Square with accum  = sum c^2
    per chunk, PIPELINED: each gather chunk is consumed the moment its
    DMA semaphore fires, so compute rides inside the gather window and
    only the last chunk's ~0.9 us remains on the tail.
  * sum x^2 runs before any gather lands (x arrives ~2.5 us earlier),
    split V/S so both engines are warm but free by chunk-0 time.
  * No PE / PSUM / on-device final reduce: the [128, 10] fp32 partial
    columns are DMA'd out per core and the host does the (blessed)
    all-reduce: loss = sum(all partials) / B.

The 4 indirect gathers stay 4 separate single-offset-column instructions:
SWDGE ucode only honors one offset per partition per instruction (a
[128, 4] offset AP writes ~1% of the destination -- probed on HW), so
128 rows x 4 is the minimum instruction count.

Per core: 512 samples; sample s lives at (partition s%128, chunk s//128),
so the gather-offset tile is lab[p, c] = labels[c*128 + p] and the x tile
is loaded with the matching (c p) f -> p c f access pattern.

Manual semaphores; no Tile exit drain (bass entry preamble clears sems).
"""

from contextlib import ExitStack

import ml_dtypes
import numpy as np

import concourse.bacc as bacc
import concourse.bass as bass
from concourse import mybir

from concourse.bass_utils import run_bass_kernel_spmd

BATCH = 4096
NUM_CLASSES = 10000
FEAT_DIM = 512
N_CORES = 8
BPC = BATCH // N_CORES   # samples per core = 512
P = 128                  # SBUF partitions
CHUNKS = BPC // P        # 4 chunks of 128 samples per core
HALF = CHUNKS * FEAT_DIM // 2
NCOL = 3 + 2 * CHUNKS    # cols: xsq_v, xsq_s1, xsq_s2, cc0-3, xc0-3

AF = mybir.AluOpType
ACTF = mybir.ActivationFunctionType
BF16 = mybir.dt.bfloat16

_NC_CACHE = {}


def _build_bass():
    nc = bacc.Bacc(None, target_bir_lowering=False, num_swdge_queues=2)

    x_in = nc.dram_tensor("x", [BPC, FEAT_DIM], BF16, kind="ExternalInput")
    lab_in = nc.dram_tensor("labels", [P, CHUNKS], mybir.dt.int32,
                            kind="ExternalInput")
    cen_in = nc.dram_tensor("centers", [NUM_CLASSES, FEAT_DIM], BF16,
                            kind="ExternalInput")
    out_t = nc.dram_tensor("out", [P, NCOL], mybir.dt.float32,
                           kind="ExternalOutput")

    with ExitStack() as ctx:
        ec = ctx.enter_context
        lab_sb = ec(nc.sbuf_tensor("lab_sb", [P, CHUNKS], mybir.dt.int32))
        xt = ec(nc.sbuf_tensor("xt", [P, CHUNKS * FEAT_DIM], BF16))
        ct = ec(nc.sbuf_tensor("ct", [P, CHUNKS * FEAT_DIM], BF16))
        # scratch for the mandatory elementwise outputs of the fused ops
        sv = ec(nc.sbuf_tensor("sv", [P, CHUNKS * FEAT_DIM], BF16))
        ss = ec(nc.sbuf_tensor("ss", [P, CHUNKS * FEAT_DIM], BF16))
        accs = ec(nc.sbuf_tensor("accs", [P, NCOL], mybir.dt.float32))
        s_lab = ec(nc.semaphore("s_lab"))
        s_x = ec(nc.semaphore("s_x"))
        s_gs = [ec(nc.semaphore(f"s_g{a}")) for a in range(CHUNKS)]
        s_done = ec(nc.semaphore("s_done"))
        s_out = ec(nc.semaphore("s_out"))

        # ---- Sync: offset tile first (gathers depend on it), then x as
        # one DMA (partition p <- rows {c*128+p}, 4 strips of 1 KB each).
        nc.sync.dma_start(out=lab_sb[:], in_=lab_in[:]).then_inc(s_lab, 16)
        nc.sync.dma_start(
            out=xt[:].rearrange("p (c f) -> p c f", c=CHUNKS),
            in_=x_in[:].rearrange("(c p) f -> p c f", p=P),
        ).then_inc(s_x, 16)

        # ---- GpSimd: the four 128-row gathers (SWDGE, ~1.1 us each).
        nc.gpsimd.wait_ge(s_lab, 16)
        for a in range(CHUNKS):
            gi = nc.gpsimd.indirect_dma_start(
                out=ct[:, a * FEAT_DIM:(a + 1) * FEAT_DIM],
                out_offset=None,
                in_=cen_in[:],
                in_offset=bass.IndirectOffsetOnAxis(
                    ap=lab_sb[:, a:a + 1], axis=0),
            )
            if a % 2 == 1:
                gi.ins.queue = "qPoolDynamic1"
            gi.then_inc(s_gs[a], 16)

        # ---- Vector: per-chunk -2 sum x.c as each gather chunk lands;
        # a [128, 1024] slice of sum x^2 sits in the g1->g2 idle gap so it
        # does not contend with SWDGE descgen (which runs ~70% slower when
        # V/S hammer SBUF during it).
        F = FEAT_DIM
        nc.vector.wait_ge(s_x, 16)
        for a in range(CHUNKS):
            sl = slice(a * F, (a + 1) * F)
            nc.vector.wait_ge(s_gs[a], 16)
            nc.vector.scalar_tensor_tensor(
                out=sv[:, sl], in0=xt[:, sl], scalar=-2.0, in1=ct[:, sl],
                op0=AF.mult, op1=AF.mult,
                accum_out=accs[:, 3 + CHUNKS + a:4 + CHUNKS + a],
            ).then_inc(s_done, 1)
            if a == 1:
                nc.vector.scalar_tensor_tensor(
                    out=sv[:, :HALF], in0=xt[:, :HALF], scalar=1.0,
                    in1=xt[:, :HALF], op0=AF.mult, op1=AF.mult,
                    accum_out=accs[:, 0:1]).then_inc(s_done, 1)

        # ---- Scalar: a short [128, 512] x^2 slice first (anchors the
        # one-time ACT_TABLE_LOAD before any gather wait, so it runs in
        # the DMA window), per-chunk sum c^2, and the last x^2 slice in
        # the same g1->g2 gap as Vector's.
        nc.scalar.wait_ge(s_x, 16)
        nc.scalar.activation(
            out=ss[:, HALF:HALF + F], in_=xt[:, HALF:HALF + F],
            func=ACTF.Square, accum_out=accs[:, 1:2]).then_inc(s_done, 1)
        for a in range(CHUNKS):
            sl = slice(a * F, (a + 1) * F)
            nc.scalar.wait_ge(s_gs[a], 16)
            nc.scalar.activation(
                out=ss[:, sl], in_=ct[:, sl], func=ACTF.Square,
                accum_out=accs[:, 3 + a:4 + a]).then_inc(s_done, 1)
            if a == 1:
                nc.scalar.activation(
                    out=ss[:, HALF + F:], in_=xt[:, HALF + F:],
                    func=ACTF.Square,
                    accum_out=accs[:, 2:3]).then_inc(s_done, 1)

        # ---- Sync: output DMA of the partial columns.  No completion
        # wait: the NRT exit barrier's per-engine Drain empties Sync's
        # HWDGE queue before execution is reported complete.
        nc.sync.wait_ge(s_done, NCOL)
        nc.sync.dma_start(out=out_t[:], in_=accs[:]).then_inc(s_out, 16)

    nc.compile()
    return nc


def get_nc():
    if "nc" not in _NC_CACHE:
        _NC_CACHE["nc"] = _build_bass()
    return _NC_CACHE["nc"]


def _idx_tile(labels_shard: np.ndarray) -> np.ndarray:
    """Gather-offset layout matching the (c p) f -> p c f x tile:
    lab[p, c] = labels[c*128 + p]."""
    return np.ascontiguousarray(labels_shard.astype(np.int32)
                                .reshape(CHUNKS, P).T)  # [128, 4]


def kernel(x, labels, centers, _run_kwargs=None):
    x = np.asarray(x, dtype=np.float32).astype(ml_dtypes.bfloat16)
    labels = np.asarray(labels).astype(np.int64)
    centers = np.asarray(centers, dtype=np.float32).astype(ml_dtypes.bfloat16)

    nc = get_nc()
    in_maps = [
        {
            "x": np.ascontiguousarray(x[c * BPC:(c + 1) * BPC]),
            "labels": _idx_tile(labels[c * BPC:(c + 1) * BPC]),
            "centers": centers,
        }
        for c in range(N_CORES)
    ]
    kwargs = _run_kwargs or {}
    out = run_bass_kernel_spmd(nc, in_maps, core_ids=list(range(N_CORES)),
                               **kwargs)
    # all-reduce the per-core partial-sum columns; mean over batch
    total = 0.0
    for r in out.results:
        total += float(r["out"].astype(np.float64).sum())
    if kwargs:
        kernel.last_run = out
    return np.asarray(total / BATCH, dtype=np.float32)

